# revision 9
# baseline (speedup 1.0000x reference)
"""Trainium2 Bass kernel for a dense transformer block (attention + ReLU FFN).

Reference computation (B=4, T=2048, C=1024, H=16, D=64):
    q,k,v = per-head projections of x;  causal softmax(q k^T / sqrt(C)) v;
    concat heads;  y = relu(out @ Wf.T + bf)

Sharding over 8 NeuronCores: core (2b+p) handles batch b with heads
[8p, 8p+8).  Attention runs causally over the full T on each core.  An
in-pair AllToAll then swaps head-halves for token-halves, so each core
runs the FFN on its 1024 tokens with all 1024 channels.  Every core runs
an identical program (one shared NEFF); only input data differs.

Compute dtype: bf16 matmul operands with fp32 PSUM accumulation
(measured l2 rel-err vs fp32 reference: ~4e-3).
"""

import os
import sys

import numpy as np
import ml_dtypes

for _p in ("/opt/trn_rl_repo", "/root/.axon_site/_ro/trn_rl_repo"):
    if os.path.isdir(_p) and _p not in sys.path:
        sys.path.append(_p)

B, T, C, H, D = 4, 2048, 1024, 16, 64
P = 128           # partitions
NCT = C // P      # 8 c-tiles
NTT = T // P      # 16 s/t-tiles
HPC = H // 2      # 8 heads per core
THALF = T // 2    # 1024 tokens per core for the FFN
SCALE = float(C) ** -0.5

bf16 = ml_dtypes.bfloat16

_CACHE = {}


def build_nc():
    import concourse.bass as bass
    import concourse.tile as tile
    from concourse import bacc, mybir

    f32 = mybir.dt.float32
    b16 = mybir.dt.bfloat16

    nc = bacc.Bacc("TRN2", target_bir_lowering=False, debug=False, num_devices=8)

    xT = nc.dram_tensor("xT", [C, T], b16, kind="ExternalInput").ap()
    wq = nc.dram_tensor("wq", [C, HPC * D], b16, kind="ExternalInput").ap()
    wk = nc.dram_tensor("wk", [C, HPC * D], b16, kind="ExternalInput").ap()
    wv = nc.dram_tensor("wv", [C, HPC * D], b16, kind="ExternalInput").ap()
    wfT = nc.dram_tensor("wfT", [C, C // 2], b16, kind="ExternalInput").ap()
    tri = nc.dram_tensor("tri", [P, P], b16, kind="ExternalInput").ap()
    biasb = nc.dram_tensor("biasb", [P, C // 2], f32, kind="ExternalInput").ap()
    y = nc.dram_tensor("y", [T, C // 2], f32, kind="ExternalOutput").ap()

    EXP = mybir.ActivationFunctionType.Exp

    with tile.TileContext(nc) as tc, \
            tc.tile_pool(name="consts", bufs=1) as consts, \
            tc.tile_pool(name="dram", bufs=1, space="DRAM") as dram:

        xT_sb = consts.tile([P, NCT, T], b16)
        wq_sb = consts.tile([P, NCT, HPC * D], b16)
        wk_sb = consts.tile([P, NCT, HPC * D], b16)
        wv_sb = consts.tile([P, NCT, HPC * D], b16)
        wfT_sb = consts.tile([P, NCT, C // 2], b16)
        tri_sb = consts.tile([P, P], b16)
        biasb_sb = consts.tile([P, C // 2], f32)
        qT_sb = consts.tile([P, HPC // 2, T], b16)
        kT_sb = consts.tile([P, HPC // 2, T], b16)
        v_sb = consts.tile([P, NTT, HPC, D + 1], b16)
        ccout_sb = consts.tile([P, 2, NCT, THALF], b16)

        cc_in0 = dram.tile([HPC * D, THALF], b16)
        cc_in1 = dram.tile([HPC * D, THALF], b16)
        cc_out0 = dram.tile([C, THALF], b16)
        cc_out1 = dram.tile([C, THALF], b16)
        cc_ins, cc_outs = [cc_in0, cc_in1], [cc_out0, cc_out1]

        # ---- load constants -------------------------------------------------
        xT_r = xT.rearrange("(ct p) t -> ct p t", p=P)
        for ct in range(NCT):
            nc.sync.dma_start(out=xT_sb[:, ct, :], in_=xT_r[ct])
        for w_ap, w_t in ((wq, wq_sb), (wk, wk_sb), (wv, wv_sb)):
            w_r = w_ap.rearrange("(ct p) m -> ct p m", p=P)
            for ct in range(NCT):
                nc.sync.dma_start(out=w_t[:, ct, :], in_=w_r[ct])
        wfT_r = wfT.rearrange("(ct p) co -> ct p co", p=P)
        for ct in range(NCT):
            nc.sync.dma_start(out=wfT_sb[:, ct, :], in_=wfT_r[ct])
        nc.sync.dma_start(out=tri_sb, in_=tri)
        nc.sync.dma_start(out=biasb_sb, in_=biasb)
        nc.vector.memset(v_sb[:, :, :, D:D + 1], 1.0)

        # ---- phase 1: QKV projections --------------------------------------
        # qT/kT: [2 heads x 64 d stacked on partitions, T] per head-pair
        # v:     [s on partitions, head, d] (+ ones column for the softmax Z)
        import contextlib
        scope = (nc.named_scope if os.environ.get("BASS_SCOPES")
                 else (lambda _n: contextlib.nullcontext()))
        with tc.tile_pool(name="qkv_ps", bufs=3, space="PSUM") as qkv_ps, \
                scope("qkv"):
            for hp in range(HPC // 2):
                for dst, w_t in ((qT_sb, wq_sb), (kT_sb, wk_sb)):
                    for g in range(T // 512):
                        ps = qkv_ps.tile([P, 512], f32, tag="qk")
                        for ct in range(NCT):
                            nc.tensor.matmul(
                                ps,
                                lhsT=w_t[:, ct, hp * P:(hp + 1) * P],
                                rhs=xT_sb[:, ct, 512 * g:512 * (g + 1)],
                                start=(ct == 0), stop=(ct == NCT - 1),
                            )
                        nc.any.tensor_copy(
                            out=dst[:, hp, 512 * g:512 * (g + 1)], in_=ps)
            for st in range(NTT):
                ps = qkv_ps.tile([P, 512], f32, tag="v")
                for ct in range(NCT):
                    nc.tensor.matmul(
                        ps,
                        lhsT=xT_sb[:, ct, P * st:P * (st + 1)],
                        rhs=wv_sb[:, ct, :],
                        start=(ct == 0), stop=(ct == NCT - 1),
                    )
                nc.any.tensor_copy(
                    out=v_sb[:, st, :, 0:D],
                    in_=ps.rearrange("p (h d) -> p h d", d=D))

        # ---- phase 2: causal attention per head ----------------------------
        # scoresT[s, t] = k_j^T q / sqrt(C) layout; exp on ACT; AV with an
        # appended ones column so row 64 of the accumulator is the softmax Z.
        with (
            tc.tile_pool(name="sc_ps", bufs=2, space="PSUM") as sc_pool,
            tc.tile_pool(name="av_ps", bufs=2, space="PSUM") as av_pool,
            tc.tile_pool(name="wt", bufs=3) as wt_pool,
            tc.tile_pool(name="norm", bufs=3) as norm_pool,
        ):
            for th in range(2):
              with scope(f"attn_th{th}"):
                for h in range(HPC):
                    hp, qh = divmod(h, 2)
                    base = 64 * qh
                    t0 = THALF * th
                    av = av_pool.tile([P, THALF], f32, tag="av")
                    jmax = 8 * th + 8
                    for j in range(jmax):
                        off = max(0, P * j - t0)
                        # 512-aligned pieces of [off, 1024); region 0 is
                        # [*, 512), region 1 is [*, 1024)
                        pieces = [(off, 512), (512, 1024)] if off < 512 \
                            else [(off, 1024)]
                        last_j = {0: 8 * th + 3, 1: jmax - 1}
                        sc = sc_pool.tile([P, THALF], f32, tag="sc")
                        for (o, e) in pieces:
                            nc.tensor.matmul(
                                sc[:, o:e],
                                lhsT=kT_sb[base:base + 64, hp, P * j:P * (j + 1)],
                                rhs=qT_sb[base:base + 64, hp, t0 + o:t0 + e],
                                start=True, stop=True,
                            )
                        wt = wt_pool.tile([P, THALF], b16, tag="wt")
                        nc.scalar.activation(
                            out=wt[:, off:THALF], in_=sc[:, off:THALF],
                            func=EXP, scale=SCALE)
                        if P * j >= t0:  # diagonal tile: causal mask
                            nc.vector.tensor_mul(
                                out=wt[:, off:off + P],
                                in0=wt[:, off:off + P], in1=tri_sb)
                        for (o, e) in pieces:
                            region = 0 if o < 512 else 1
                            nc.tensor.matmul(
                                av[0:D + 1, o:e],
                                lhsT=v_sb[:, j, h, :],
                                rhs=wt[:, o:e],
                                start=(j == 0), stop=(j == last_j[region]),
                            )
                    # normalize: u[d, t] / Z[t]; Z is row 64 of av.
                    # (copy Z to partition 0 first: the custom-DVE recip
                    # mishandles a nonzero base partition on its input)
                    zrow = norm_pool.tile([1, THALF], f32, tag="zrow")
                    nc.any.tensor_copy(out=zrow, in_=av[D:D + 1, 0:THALF])
                    zr = norm_pool.tile([1, THALF], f32, tag="zr")
                    nc.vector.reciprocal_approx_fast(out=zr, in_=zrow)
                    zb = norm_pool.tile([64, THALF], f32, tag="zb")
                    zr_b = bass.AP(
                        tensor=zr.tensor, offset=zr.offset,
                        ap=[list(zr.ap[0]), [0, 64], [1, THALF]])
                    nc.gpsimd.dma_start(out=zb, in_=zr_b)
                    stage = norm_pool.tile([64, THALF], b16, tag="stage")
                    nc.vector.tensor_mul(
                        out=stage, in0=av[0:64, 0:THALF], in1=zb)
                    nc.sync.dma_start(
                        out=cc_ins[th][64 * h:64 * (h + 1), :],
                        in_=stage)
                # pair AllGather of this t-half: [512, 1024] -> [1024, 1024]
                nc.gpsimd.collective_compute(
                    "AllGather",
                    mybir.AluOpType.bypass,
                    replica_groups=[[0, 1], [2, 3], [4, 5], [6, 7]],
                    ins=[cc_ins[th].opt()],
                    outs=[cc_outs[th].opt()],
                )
                cc_out_r = cc_outs[th].rearrange("(ci p) t -> ci p t", p=P)
                for ci in range(NCT):
                    nc.sync.dma_start(
                        out=ccout_sb[:, th, ci, :], in_=cc_out_r[ci])

        # ---- phase 4: FFN, all tokens x this core's 512 output channels ----
        y_r = y.rearrange("(tt p) co -> tt p co", p=P)
        with (
            tc.tile_pool(name="ffn_ps", bufs=3, space="PSUM") as ffn_pool,
            tc.tile_pool(name="yout", bufs=3) as y_pool,
        ):
            for tt in range(NTT):
              with scope(f"ffn_t{tt//8}"):
                th, tl = divmod(tt, NTT // 2)
                ps = ffn_pool.tile([P, C // 2], f32, tag="ffn")
                for ci in range(NCT):
                    nc.tensor.matmul(
                        ps,
                        lhsT=ccout_sb[:, th, ci, P * tl:P * (tl + 1)],
                        rhs=wfT_sb[:, ci, :],
                        start=(ci == 0), stop=(ci == NCT - 1),
                    )
                ysb = y_pool.tile([P, C // 2], f32, tag="y")
                nc.vector.tensor_add(out=ysb, in0=ps, in1=biasb_sb)
                nc.vector.tensor_scalar_max(out=ysb, in0=ysb, scalar1=0.0)
                nc.sync.dma_start(out=y_r[tt], in_=ysb)

    nc.compile()
    return nc


def make_in_maps(x, Wq, Wk, Wv, Wf, bf):
    x = np.asarray(x, np.float32)
    tri_m = np.ascontiguousarray(np.triu(np.ones((P, P), np.float32))).astype(bf16)
    bf_f = np.asarray(bf, np.float32)
    wfT_f = np.asarray(Wf, np.float32).T
    in_maps = []
    for core in range(8):
        b, p = divmod(core, 2)
        sl = slice(HPC * p, HPC * (p + 1))
        in_maps.append({
            "xT": np.ascontiguousarray(x[b].T).astype(bf16),
            "wq": np.ascontiguousarray(
                np.asarray(Wq, np.float32)[:, sl].reshape(C, HPC * D)).astype(bf16),
            "wk": np.ascontiguousarray(
                np.asarray(Wk, np.float32)[:, sl].reshape(C, HPC * D)).astype(bf16),
            "wv": np.ascontiguousarray(
                np.asarray(Wv, np.float32)[:, sl].reshape(C, HPC * D)).astype(bf16),
            "wfT": np.ascontiguousarray(
                wfT_f[:, 512 * p:512 * (p + 1)]).astype(bf16),
            "tri": tri_m,
            "biasb": np.ascontiguousarray(np.tile(
                bf_f[None, 512 * p:512 * (p + 1)], (P, 1))),
        })
    return in_maps


def run(x, Wq, Wk, Wv, Wf, bf, trace=False, **spmd_kwargs):
    from concourse.bass_utils import run_bass_kernel_spmd

    if "nc" not in _CACHE:
        _CACHE["nc"] = build_nc()
    nc = _CACHE["nc"]
    in_maps = make_in_maps(x, Wq, Wk, Wv, Wf, bf)
    res = run_bass_kernel_spmd(
        nc, in_maps, core_ids=list(range(8)), trace=trace, **spmd_kwargs)
    out = np.zeros((B, T, C), np.float32)
    for core in range(8):
        b, p = divmod(core, 2)
        out[b, :, 512 * p:512 * (p + 1)] = res.results[core]["y"]
    return out, res


def kernel(x, Wq, Wk, Wv, Wf, bf):
    out, _ = run(x, Wq, Wk, Wv, Wf, bf, trace=False)
    return out
